# revision 2
# baseline (speedup 1.0000x reference)
"""FlowNet-C correlation layer (MAX_DISP=20, STRIDE=2) on 8 trn2 cores, v3.

Sharding: core k handles batch b=k//2, output-row half k%2 (24 output rows).

Per output row oh (full-res rows h0=2oh, h0+1), the C=128 contraction runs
on the TensorEngine as banded-Gram matmuls over w'-blocks: for each block
of 12 padded columns, rhs streams (w' outer, dy inner) so one matmul of
N=12*41=492 produces G[w, w', dy] for all 41 row-offsets dy at once, in
(w', dy)-contiguous PSUM layout (h-pair pooled via PSUM accumulation).
x2p is pre-transposed on the host to [C, WP, ROWS] so that this streaming
order is unit-stride in SBUF (contiguous dy runs of 41) and the PE fetches
columns at full rate.
PSUM evacuation to bf16 SBUF is then a unit-stride cast (alternating
DVE/ACT).  The assembled [96, 136*41] Gram is dumped flat to DRAM where
the diagonal band S[w, dx*41+dy] = G[w, w+dx, dy] lives at
w*5617 + (dx*41+dy) -- one contiguous 1681-element run per partition --
so extraction is a single strided read per w-parity; the 2x2 pool
finishes with one on-chip bf16 add.  The Gram dump rides the scalar-engine
HWDGE ring while gathers/outputs ride the sync ring, so the two DMA
streams drain in parallel.  bf16 in/out; scale 1/(4*C) folded into
x1 on the host; final bf16->f32 upcast on the host.
"""

import os

import ml_dtypes
import numpy as np

import concourse.bacc as bacc
import concourse.bass as bass
import concourse.mybir as mybir
import concourse.tile as tile
from concourse.ap import AP
from concourse.bass import MemorySpace
from concourse.bass_utils import run_bass_kernel_spmd

MD = 20
K = 41
CC = K * K            # 1681
B, C, H, W = 4, 128, 96, 96
OH, OW = 48, 48
WP = W + 2 * MD       # 136
HH = 48               # full-res rows per core
NOH = 24              # output rows per core
ROWS = HH + K         # 89 x2p rows needed per core
DR = WP * K           # 5576: dump row stride (per w)
SH = DR + K           # 5617: shear read stride (per w)
WB = 12               # w'-block size (N = 12*41 = 492 <= 512 psum bank)

F32 = mybir.dt.float32
BF16 = mybir.dt.bfloat16

X2T = True  # x2p host-transposed to [C, WP, ROWS]
BLOCKS = [(b0, min(WB, WP - b0)) for b0 in range(0, WP, WB)]  # 11x12 + 1x4

LAST_EXEC_NS = None
_CACHED = None


def _build_nc():
    nc = bacc.Bacc("TRN2", target_bir_lowering=False)
    x1d = nc.dram_tensor("x1h", [C, HH * W], BF16, kind="ExternalInput")
    x2d = nc.dram_tensor("x2p", [C, WP * ROWS], BF16, kind="ExternalInput")
    outd = nc.dram_tensor("out", [NOH * OW, CC], BF16, kind="ExternalOutput")

    with tile.TileContext(nc) as tc:
        with (
            tc.tile_pool(name="inp", bufs=1) as inp_pool,
            tc.tile_pool(name="gs", bufs=3) as gs_pool,
            tc.tile_pool(name="st", bufs=3) as s_pool,
            tc.tile_pool(name="ps", bufs=8, space=MemorySpace.PSUM) as psum_pool,
            tc.tile_pool(name="dr", bufs=6, space=MemorySpace.DRAM) as dram_pool,
        ):
            A = inp_pool.tile([C, HH * W], BF16)
            Bt = inp_pool.tile([C, WP, ROWS], BF16)
            # TRN2 ldweights encodes only ONE semaphore wait, so matmuls must
            # only ever depend on a single sem.  Loads therefore bounce
            # through staging tiles and a DVE copy instead of DMAing into
            # A/Bt directly.
            with tc.tile_pool(name="stg", bufs=3) as stage_pool:
                for a0 in range(0, HH, 12):
                    stg = stage_pool.tile([C, 12 * W], BF16, tag="stg")
                    nc.sync.dma_start(stg[:], x1d[:, a0 * W:(a0 + 12) * W])
                    nc.vector.tensor_copy(A[:, a0 * W:(a0 + 12) * W], stg[:])
                for w0 in range(0, WP, 12):
                    w1 = min(w0 + 12, WP)
                    stg = stage_pool.tile([C, 12 * ROWS], BF16, tag="stg")
                    nc.sync.dma_start(stg[:, :(w1 - w0) * ROWS],
                                      x2d[:, w0 * ROWS:w1 * ROWS])
                    nc.vector.tensor_copy(Bt[:, w0:w1, :],
                                          stg[:, :(w1 - w0) * ROWS])

            for oh in range(NOH):
                h0 = 2 * oh
                gs = gs_pool.tile([W, DR], BF16, tag="gs")
                for bi, (b0, bw) in enumerate(BLOCKS):
                    n = bw * K
                    ps = psum_pool.tile([W, WB * K], F32, tag="ps")
                    # rhs streams (w' outer, dy inner): column (w', dy) =
                    # x2pT[:, b0+w', h+j+dy] -- unit-stride dy runs
                    for j in range(2):
                        rhs = Bt[:, b0:b0 + bw, h0 + j:h0 + j + K]
                        nc.tensor.matmul(
                            ps[:, :n],
                            A[:, (h0 + j) * W:(h0 + j + 1) * W],
                            rhs,
                            start=(j == 0), stop=(j == 1),
                        )
                    # unit-stride PSUM (w', dy) -> gs[w, b0*41 : ...]
                    dst = gs[:, b0 * K:(b0 + bw) * K]
                    if bi % 2 == 0:
                        nc.vector.tensor_copy(dst, ps[:, :n])
                    else:
                        nc.scalar.copy(dst, ps[:, :n])
                # dump the whole-oh Gram: flat [w*5576 + w'*41 + dy]
                # (scalar-engine HWDGE ring, parallel to the sync ring)
                dscr = dram_pool.tile([W, DR], BF16, tag="dscr")
                nc.scalar.dma_start(dscr[:], gs[:])
                dt_ = dscr[:].tensor
                # shear gather: S[ow, d] = dump[2ow*5617 + d] (+ odd row)
                Se = s_pool.tile([OW, CC], BF16, tag="se")
                So = s_pool.tile([OW, CC], BF16, tag="so")
                nc.sync.dma_start(Se[:], AP(dt_, 0, [[2 * SH, OW], [1, CC]]))
                nc.sync.dma_start(So[:], AP(dt_, SH, [[2 * SH, OW], [1, CC]]))
                # 2x2-pool finish: one unit-stride bf16 add
                S = s_pool.tile([OW, CC], BF16, tag="s")
                nc.vector.tensor_add(S[:], Se[:], So[:])
                nc.sync.dma_start(outd[oh * OW:(oh + 1) * OW, :], S[:])
    nc.compile()
    return nc


def kernel(x1: np.ndarray, x2: np.ndarray) -> np.ndarray:
    global LAST_EXEC_NS, _CACHED
    x1 = (np.asarray(x1, dtype=np.float32) * np.float32(1.0 / (4 * C))).astype(
        ml_dtypes.bfloat16)
    x2 = np.asarray(x2, dtype=np.float32).astype(ml_dtypes.bfloat16)
    x2p = np.zeros((B, C, HH + ROWS, WP), dtype=ml_dtypes.bfloat16)
    x2p[:, :, MD:MD + H, MD:MD + W] = x2

    if _CACHED is None:
        _CACHED = _build_nc()
    nc = _CACHED

    in_maps = []
    for core in range(8):
        b, half = core // 2, core % 2
        a = np.ascontiguousarray(
            x1[b, :, half * HH:(half + 1) * HH, :].reshape(C, HH * W))
        x2s = np.ascontiguousarray(
            x2p[b, :, half * HH:half * HH + ROWS, :].transpose(0, 2, 1)
            .reshape(C, WP * ROWS))
        in_maps.append({"x1h": a, "x2p": x2s})

    res = run_bass_kernel_spmd(
        nc, in_maps, core_ids=list(range(8)),
        trace=os.environ.get("CORR_TRACE") == "1",
    )
    LAST_EXEC_NS = res.exec_time_ns

    out = np.empty((B, CC, OH, OW), dtype=np.float32)
    for core in range(8):
        b, half = core // 2, core % 2
        r = res.results[core]["out"].astype(np.float32).reshape(NOH, OW, CC)
        out[b, :, half * NOH:(half + 1) * NOH, :] = r.transpose(2, 0, 1)
    return out
